# revision 10
# baseline (speedup 1.0000x reference)
"""Trainium2 Bass kernel for MAGNN link prediction (nn_MAGNN_lp) — v2.

Differences from v1 (dump + regather): the segment softmax/sum is computed
with one-hot matmuls into PSUM accumulators, eliminating the 13MB-per-
metapath row dump to DRAM and the per-target regather entirely.

Sharding: B=8192 targets across 8 cores (1024 each); instances sharded by
target range so aggregation is core-local. Node towers sharded by rows
(5000/core), projected node table ([40000, 128] bf16, vec + zero pad per
row) AllGathered in DRAM. Instances are packed into 128-instance tiles
grouped by (lo/hi gather class, target slice of 128); each tile is
slice-pure, so its contributions accumulate into one of 8 PSUM banks via
matmul(onehot[inst,tgt]^T @ [w*eft | w]). The one-hot is built on DVE by
comparing an iota row against host-shifted local target ids. After each
metapath: normalize + ELU into SBUF; then semantic attention (AllReduce of
4 scalars), product MLP, 2-way softmax.
"""
import math
from dataclasses import dataclass

import numpy as np

import concourse.bass as bass
import concourse.mybir as mybir
import concourse.tile as tile
from concourse import bacc
from concourse.masks import make_identity

F32 = mybir.dt.float32
BF16 = mybir.dt.bfloat16
I32 = mybir.dt.int32
I16 = mybir.dt.int16
AF = mybir.ActivationFunctionType
ALU = mybir.AluOpType
PSUM = "PSUM"


@dataclass
class Cfg:
    n_cores: int = 8
    B: int = 8192
    HID: int = 64
    H: int = 8
    D: int = 8
    F0: int = 512
    AV: int = 128
    CH: int = 128
    n_nodes: int = 40000
    LO: int = 32768        # lo/hi table split (int16 gather index limit)
    T: int = 260           # 128-instance tiles per metapath per core
    Tc: int = 20           # tiles per processing chunk
    n_mp: int = 4
    RW: int = 72           # row width: 64 weighted values + 8 exp weights
    gelu: bool = True      # False: Tanh stand-in (CoreSim lacks Gelu)
    shared_table: bool = True
    tiles_cs: np.ndarray | None = None   # [n_mp, 8 cls, 8 slc]
    skip_gather: bool = False
    skip_mm: bool = False
    skip_pe: bool = False
    skip_vec: bool = False
    skip_semantic: bool = False
    skip_allgather: bool = False
    skip_tower: bool = False
    force_Tc: int = 20

    @property
    def B_loc(self):
        return self.B // self.n_cores

    @property
    def nodes_core(self):
        return self.n_nodes // self.n_cores

    @property
    def node_tiles(self):
        return (self.nodes_core + 127) // 128

    @property
    def E_loc(self):
        return self.T * 128

    @property
    def n_chunks(self):
        return self.T // self.Tc

    @property
    def kF(self):
        return self.F0 // 128

    @property
    def n_slc(self):
        return self.B_loc // 128


def _ap_with(ap, offset_delta, tail_dims):
    """Copy an AP, keeping its partition dim, replacing trailing free dims."""
    return bass.AP(ap.tensor, ap.offset + offset_delta,
                   [list(ap.ap[0])] + [list(d) for d in tail_dims])


def _class_segments(tiles_per_class):
    segs, t = [], 0
    for cls in range(8):
        n = int(tiles_per_class[cls])
        if n:
            segs.append((cls, t, t + n))
            t += n
    return segs


def _gather_calls(tiles_per_class, Tc, n_chunks):
    """calls[ch][l] = [(toff_rel, ntiles, hi)], adjacent same-hi merged."""
    segs = _class_segments(tiles_per_class)
    calls = []
    for ch in range(n_chunks):
        c0, c1 = ch * Tc, (ch + 1) * Tc
        per_l = []
        for l in range(3):
            lst = []
            for cls, s0, s1 in segs:
                a, b = max(c0, s0), min(c1, s1)
                if a >= b:
                    continue
                hi = bool((cls >> l) & 1)
                if lst and lst[-1][2] == hi and lst[-1][0] + lst[-1][1] == a - c0:
                    lst[-1] = (lst[-1][0], lst[-1][1] + (b - a), hi)
                else:
                    lst.append((a - c0, b - a, hi))
            per_l.append(lst)
        calls.append(per_l)
    return calls


def _slice_map(tcs_mp):
    """tile index -> slice, plus first/last tile per slice."""
    slice_of = []
    for cl in range(8):
        for s in range(8):
            slice_of += [s] * int(tcs_mp[cl][s])
    first, last = {}, {}
    for t, s in enumerate(slice_of):
        if s not in first:
            first[s] = t
        last[s] = t
    return slice_of, first, last


def _finish(nc):
    nc.compile()
    return nc


def build_program(cfg: Cfg):
    c = cfg
    assert c.tiles_cs is not None
    nc = bacc.Bacc("TRN2", target_bir_lowering=False, debug=False,
                   num_devices=c.n_cores)

    def di(name, shape, dtype=F32):
        return nc.dram_tensor(name, list(shape), dtype, kind="ExternalInput")

    T8 = c.T * 8
    feats = di("feats", (c.node_tiles * 128, c.F0))
    pw = di("pw", (c.F0, c.HID))
    pb = di("pb", (c.HID,))
    w2 = di("w2", (c.HID, c.HID))
    b2 = di("b2", (c.HID,))
    g = di("g", (c.HID,))
    be = di("be", (c.HID,))
    rvec = di("rvec", (c.HID,))
    attn = di("attn", (c.n_mp, c.HID))
    emi16 = di("emi16", (c.n_mp * 3 * 128, T8), I16)
    tlocs = di("tlocs", (c.n_mp * 128, c.T))
    suw1 = di("suw1", (c.HID, c.AV))
    sub1 = di("sub1", (c.AV,))
    suw2 = di("suw2", (c.AV,))
    siw1 = di("siw1", (c.HID, c.AV))
    sib1 = di("sib1", (c.AV,))
    siw2 = di("siw2", (c.AV,))
    cw1 = di("cw1", (c.HID, c.CH))
    cb1 = di("cb1", (c.CH,))
    cw2 = di("cw2", (c.CH, 2))
    outd = nc.dram_tensor("out", [c.B_loc, 2], F32, kind="ExternalOutput")

    HID, H, D = c.HID, c.H, c.D
    NPAIR = HID // 2

    with tile.TileContext(nc) as tc:
        with (
            tc.tile_pool(name="const", bufs=1) as kpool,
            tc.tile_pool(name="dram", bufs=1, space="DRAM") as dpool,
        ):
            pk_ctx = tc.tile_pool(name="ps_const", bufs=1, space="PSUM")
            pkpool = pk_ctx.__enter__()
            # ---------- constants ----------
            id128 = kpool.tile([128, 128], F32, tag="id128")
            make_identity(nc, id128[:])
            ones1 = kpool.tile([1, 128], F32, tag="ones1")
            nc.vector.memset(ones1[:], 1.0)
            onescol = kpool.tile([128, 1], F32, tag="onescol")
            nc.vector.memset(onescol[:], 1.0)
            epscol = kpool.tile([128, 1], F32, tag="epscol")
            nc.vector.memset(epscol[:], 1e-5)
            iotaI = kpool.tile([128, 128], I32, tag="iotaI")
            nc.gpsimd.iota(iotaI[:], pattern=[[1, 128]], base=0,
                           channel_multiplier=0)
            iotaF = kpool.tile([128, 128], F32, tag="iotaF")
            nc.vector.tensor_copy(iotaF[:], iotaI[:])

            def rep_row(dram_vec, n, scale=None, tag=None):
                row = kpool.tile([1, n], F32, tag=f"{tag}_row")
                nc.sync.dma_start(row[:], dram_vec)
                ps = pkpool.tile([128, 512], F32, space=PSUM, tag="reppsum")
                nc.tensor.matmul(out=ps[:, :n], lhsT=ones1[:], rhs=row[:],
                                 start=True, stop=True)
                rep = kpool.tile([128, n], F32, tag=tag)
                if scale is None:
                    nc.vector.tensor_copy(rep[:], ps[:, :n])
                else:
                    nc.vector.tensor_scalar_mul(rep[:], ps[:, :n], scale)
                return rep

            def vrow(x):
                return x.ap().rearrange("(o a) -> o a", o=1)

            PBrep = rep_row(vrow(pb), HID, tag="PBrep")
            B2rep = rep_row(vrow(b2), HID, tag="B2rep")
            G3rep = rep_row(vrow(g), HID, scale=1.0 / 3.0, tag="G3rep")
            BE3rep = rep_row(vrow(be), HID, scale=1.0 / 3.0, tag="BE3rep")
            SUB1rep = rep_row(vrow(sub1), c.AV, tag="SUB1rep")
            SIB1rep = rep_row(vrow(sib1), c.AV, tag="SIB1rep")
            SUW2rep = rep_row(vrow(suw2), c.AV, tag="SUW2rep")
            SIW2rep = rep_row(vrow(siw2), c.AV, tag="SIW2rep")
            CB1rep = rep_row(vrow(cb1), c.CH, tag="CB1rep")
            CW20rep = rep_row(cw2.ap()[:, 0:1].rearrange("a o -> o a"), c.CH, tag="CW20rep")
            CW21rep = rep_row(cw2.ap()[:, 1:2].rearrange("a o -> o a"), c.CH, tag="CW21rep")
            ATTNrep = [rep_row(attn.ap()[mp:mp + 1, :], HID, tag=f"ATTN{mp}")
                       for mp in range(c.n_mp)]

            # ---------- rotation constants (normalize r0 on device) ----------
            rcol = kpool.tile([HID, 1], F32, tag="rcol")
            nc.sync.dma_start(rcol[:], rvec.ap().rearrange("(p o) -> p o", o=1))
            idh = kpool.tile([HID, HID], F32, tag="idh")
            make_identity(nc, idh[:])
            Sp = kpool.tile([HID, HID], F32, tag="Sp")
            nc.vector.memset(Sp[:], 0.0)
            nc.vector.tensor_copy(Sp[:, 1:HID], idh[:, 0:HID - 1])
            Sm = kpool.tile([HID, HID], F32, tag="Sm")
            nc.vector.memset(Sm[:], 0.0)
            nc.vector.tensor_copy(Sm[:, 0:HID - 1], idh[:, 1:HID])
            pidx = kpool.tile([HID, 1], I32, tag="pidx")
            nc.gpsimd.iota(pidx[:], pattern=[[0, 1]], base=0, channel_multiplier=1)
            podd_i = kpool.tile([HID, 1], I32, tag="podd_i")
            nc.vector.tensor_scalar(podd_i[:], pidx[:], 1, None, ALU.bitwise_and)
            podd = kpool.tile([HID, 1], F32, tag="podd")
            nc.vector.tensor_copy(podd[:], podd_i[:])
            peven = kpool.tile([HID, 1], F32, tag="peven")
            nc.vector.tensor_scalar(peven[:], podd[:], -1.0, -1.0, ALU.add, ALU.mult)
            Spe = kpool.tile([HID, HID], F32, tag="Spe")
            nc.vector.tensor_scalar_mul(Spe[:], Sp[:], peven[:])
            Smo = kpool.tile([HID, HID], F32, tag="Smo")
            nc.vector.tensor_scalar_mul(Smo[:], Sm[:], podd[:])
            Ie = kpool.tile([HID, HID], F32, tag="Ie")
            nc.vector.tensor_scalar_mul(Ie[:], idh[:], peven[:])
            Io = kpool.tile([HID, HID], F32, tag="Io")
            nc.vector.tensor_scalar_mul(Io[:], idh[:], podd[:])
            M2 = kpool.tile([HID, HID], F32, tag="M2")
            nc.vector.tensor_tensor(M2[:], idh[:], Spe[:], ALU.add)
            nc.vector.tensor_tensor(M2[:], M2[:], Smo[:], ALU.add)
            Me = kpool.tile([HID, HID], F32, tag="Me")
            nc.vector.tensor_tensor(Me[:], Ie[:], Spe[:], ALU.add)
            Mo = kpool.tile([HID, HID], F32, tag="Mo")
            nc.vector.tensor_tensor(Mo[:], Io[:], Smo[:], ALU.add)
            sqc = kpool.tile([HID, 1], F32, tag="sqc")
            nc.vector.tensor_tensor(sqc[:], rcol[:], rcol[:], ALU.mult)
            n2 = pkpool.tile([HID, 1], F32, space=PSUM, tag="n2")
            nc.tensor.matmul(out=n2[:], lhsT=M2[:], rhs=sqc[:], start=True, stop=True)
            nrm = kpool.tile([HID, 1], F32, tag="nrm")
            nc.scalar.activation(nrm[:], n2[:], AF.Sqrt)
            invn = kpool.tile([HID, 1], F32, tag="invn")
            nc.vector.reciprocal(invn[:], nrm[:])
            rn = kpool.tile([HID, 1], F32, tag="rn")
            nc.vector.tensor_scalar_mul(rn[:], rcol[:], invn[:])
            cr2 = pkpool.tile([HID, 1], F32, space=PSUM, tag="cr2")
            nc.tensor.matmul(out=cr2[:], lhsT=Me[:], rhs=rn[:], start=True, stop=True)
            ci2 = pkpool.tile([HID, 1], F32, space=PSUM, tag="ci2")
            nc.tensor.matmul(out=ci2[:], lhsT=Mo[:], rhs=rn[:], start=True, stop=True)
            cr2s = kpool.tile([HID, 1], F32, tag="cr2s")
            nc.vector.tensor_copy(cr2s[:], cr2[:])
            ci2s = kpool.tile([HID, 1], F32, tag="ci2s")
            nc.vector.tensor_copy(ci2s[:], ci2[:])
            crrow_ps = pkpool.tile([1, HID], F32, space=PSUM, tag="crrow_ps")
            nc.tensor.matmul(out=crrow_ps[:], lhsT=cr2s[:], rhs=idh[:], start=True, stop=True)
            crrow = kpool.tile([1, HID], F32, tag="crrow")
            nc.vector.tensor_copy(crrow[:], crrow_ps[:])
            cirow_ps = pkpool.tile([1, HID], F32, space=PSUM, tag="cirow_ps")
            nc.tensor.matmul(out=cirow_ps[:], lhsT=ci2s[:], rhs=idh[:], start=True, stop=True)
            cirow = kpool.tile([1, HID], F32, tag="cirow")
            nc.vector.tensor_copy(cirow[:], cirow_ps[:])
            fidx = kpool.tile([1, HID], I32, tag="fidx")
            nc.gpsimd.iota(fidx[:], pattern=[[1, HID]], base=0, channel_multiplier=0)
            fodd_i = kpool.tile([1, HID], I32, tag="fodd_i")
            nc.vector.tensor_scalar(fodd_i[:], fidx[:], 1, None, ALU.bitwise_and)
            fsign = kpool.tile([1, HID], F32, tag="fsign")
            nc.vector.tensor_copy(fsign[:], fodd_i[:])
            nc.vector.tensor_scalar(fsign[:], fsign[:], -2.0, 1.0, ALU.mult, ALU.add)
            c2urow = kpool.tile([1, HID], F32, tag="c2urow")
            nc.vector.tensor_tensor(c2urow[:], cirow[:], fsign[:], ALU.mult)
            c2irow = kpool.tile([1, HID], F32, tag="c2irow")
            nc.vector.tensor_scalar_mul(c2irow[:], c2urow[:], -1.0)

            def rep_from_row(row, n, tag, dtype=F32):
                ps = pkpool.tile([128, 512], F32, space=PSUM, tag="reppsum")
                nc.tensor.matmul(out=ps[:, :n], lhsT=ones1[:], rhs=row[:],
                                 start=True, stop=True)
                rep = kpool.tile([128, n], dtype, tag=tag)
                nc.vector.tensor_copy(rep[:], ps[:, :n])
                return rep

            C1b = rep_from_row(crrow, HID, "C1b", BF16)
            C2b = [rep_from_row(c2urow, HID, "C2ub", BF16),
                   rep_from_row(c2irow, HID, "C2ib", BF16)]
            ATTNb = []
            for mp in range(c.n_mp):
                ab = kpool.tile([128, HID], BF16, tag=f"ATTNb{mp}")
                nc.vector.tensor_copy(ab[:], ATTNrep[mp][:])
                ATTNb.append(ab)

            pwsb = kpool.tile([128, c.kF, HID], F32, tag="pwsb")
            nc.sync.dma_start(pwsb[:], pw.ap().rearrange("(a p) c -> p a c", p=128))
            w2sb = kpool.tile([HID, HID], F32, tag="w2sb")
            nc.sync.dma_start(w2sb[:], w2.ap())
            suw1sb = kpool.tile([HID, c.AV], F32, tag="suw1sb")
            nc.sync.dma_start(suw1sb[:], suw1.ap())
            siw1sb = kpool.tile([HID, c.AV], F32, tag="siw1sb")
            nc.sync.dma_start(siw1sb[:], siw1.ap())
            cw1sb = kpool.tile([HID, c.CH], F32, tag="cw1sb")
            nc.sync.dma_start(cw1sb[:], cw1.ap())

            pk_ctx.__exit__(None, None, None)

            # ---------- tower (bf16 table rows: [vec(64) | zeros(64)]) ----------
            tower_t = dpool.tile([c.nodes_core, 128], BF16, tag="tower")
            table_t = dpool.tile(
                [c.n_nodes, 128], BF16, tag="table",
                addr_space="Shared" if c.shared_table else "Local")
            with (
                tc.tile_pool(name="tw_x", bufs=2) as xpool,
                tc.tile_pool(name="tw_ps", bufs=1, space="PSUM") as tpspool,
                tc.tile_pool(name="tw_s", bufs=3) as tspool,
                tc.tile_pool(name="tw_keep", bufs=1) as tkeep,
            ):
                if c.skip_tower:
                    zb16 = tkeep.tile([128, 128], BF16, tag="zb16")
                    nc.vector.memset(zb16[:], 0.5)
                    for j in range(c.node_tiles):
                        rows_n = min(128, c.nodes_core - j * 128)
                        nc.sync.dma_start(tower_t[j * 128:j * 128 + rows_n, :],
                                          zb16[:rows_n, :])
                # pass 1: all-Gelu; buffer centered activations + variances
                if not c.skip_tower:
                    ycs = tkeep.tile([128, c.node_tiles, HID], F32, tag="ycs")
                    vvs = tkeep.tile([128, c.node_tiles], F32, tag="vvs")
                for j in range(c.node_tiles if not c.skip_tower else 0):
                    xt = xpool.tile([128, c.F0], F32, tag="xt")
                    nc.sync.dma_start(xt[:], feats.ap()[j * 128:(j + 1) * 128, :])
                    xT = xpool.tile([128, c.kF, 128], F32, tag="xT")
                    for kk in range(c.kF):
                        pst = tpspool.tile([128, 128], F32, space=PSUM, tag="pst")
                        nc.tensor.transpose(pst[:], xt[:, kk * 128:(kk + 1) * 128], id128[:])
                        nc.vector.tensor_copy(xT[:, kk, :], pst[:])
                    z = tpspool.tile([128, HID], F32, space=PSUM, tag="z")
                    for kk in range(c.kF):
                        nc.tensor.matmul(out=z[:], lhsT=xT[:, kk, :], rhs=pwsb[:, kk, :],
                                         start=(kk == 0), stop=(kk == c.kF - 1))
                    zb = tspool.tile([128, HID], F32, tag="zb")
                    nc.vector.tensor_tensor(zb[:], z[:], PBrep[:], ALU.add)
                    h = tspool.tile([128, HID], F32, tag="h")
                    nc.scalar.activation(h[:], zb[:], AF.Gelu if c.gelu else AF.Tanh)
                    hT_ps = tpspool.tile([HID, 128], F32, space=PSUM, tag="hT_ps")
                    nc.tensor.transpose(hT_ps[:], h[:], id128[:])
                    hT = tspool.tile([HID, 128], F32, tag="hT")
                    nc.vector.tensor_copy(hT[:], hT_ps[:])
                    y = tpspool.tile([128, HID], F32, space=PSUM, tag="y")
                    nc.tensor.matmul(out=y[:], lhsT=hT[:], rhs=w2sb[:], start=True, stop=True)
                    ys = tspool.tile([128, HID], F32, tag="ys")
                    nc.vector.tensor_tensor(ys[:], y[:], B2rep[:], ALU.add)
                    nc.vector.tensor_tensor(ys[:], ys[:], zb[:], ALU.add)
                    mu = tspool.tile([128, 1], F32, tag="mu")
                    nc.vector.tensor_reduce(mu[:], ys[:], mybir.AxisListType.X, ALU.add)
                    nc.vector.tensor_scalar_mul(mu[:], mu[:], 1.0 / HID)
                    yc = ycs[:, j, :]
                    nc.vector.tensor_scalar(yc, ys[:], mu[:], None, ALU.subtract)
                    sq = tspool.tile([128, HID], F32, tag="sq")
                    nc.vector.tensor_tensor(sq[:], yc, yc, ALU.mult)
                    nc.vector.tensor_reduce(vvs[:, j:j + 1], sq[:],
                                            mybir.AxisListType.X, ALU.add)
                # pass 2: one Sqrt for all tiles, then scale + write
                if not c.skip_tower:
                    sdv = tkeep.tile([128, c.node_tiles], F32, tag="sdv")
                    nc.scalar.activation(sdv[:], vvs[:], AF.Sqrt, bias=epscol[:],
                                         scale=1.0 / HID)
                    invs = tkeep.tile([128, c.node_tiles], F32, tag="invs")
                    nc.vector.reciprocal(invs[:], sdv[:])
                for j in range(c.node_tiles if not c.skip_tower else 0):
                    tbl = tspool.tile([128, 128], BF16, tag="tbl")
                    nc.vector.memset(tbl[:, HID:128], 0.0)
                    tn = tspool.tile([128, HID], F32, tag="tn")
                    nc.vector.tensor_scalar_mul(tn[:], ycs[:, j, :], invs[:, j:j + 1])
                    nc.vector.tensor_tensor(tn[:], tn[:], G3rep[:], ALU.mult)
                    nc.vector.tensor_tensor(tbl[:, 0:HID], tn[:], BE3rep[:], ALU.add)
                    rows_n = min(128, c.nodes_core - j * 128)
                    nc.sync.dma_start(tower_t[j * 128:j * 128 + rows_n, :], tbl[:rows_n, :])

            if not c.skip_allgather:
                nc.gpsimd.collective_compute(
                    "AllGather", ALU.bypass,
                    replica_groups=[list(range(c.n_cores))],
                    ins=[tower_t.opt()], outs=[table_t.opt()],
                )

            # ---------- metapaths: gather, rotate, logits, onehot-matmul ----------
            outs_all = None
            with (
                tc.tile_pool(name="mp_idx", bufs=2) as ipool,
                tc.tile_pool(name="mp_ed", bufs=2) as edpool,
                tc.tile_pool(name="mp_row", bufs=2) as rowpool,
                tc.tile_pool(name="mp_oh", bufs=2) as ohpool,
                tc.tile_pool(name="mp_tmp", bufs=2) as mtpool,
                tc.tile_pool(name="mp_keep", bufs=1) as keep,
                tc.tile_pool(name="mp_hs", bufs=3) as hpool,
            ):
                outs_all = keep.tile([128, c.n_mp, c.n_slc, HID], F32, tag="outs_all")
                for mp in range(c.n_mp):
                    side = 0 if mp < 2 else 1
                    tcs_mp = c.tiles_cs[mp]
                    calls = _gather_calls(tcs_mp.sum(axis=1), c.Tc, c.n_chunks)
                    slice_of, first, last = _slice_map(tcs_mp)
                    emi_sb = ipool.tile([128, 3, T8], I16, tag="emi_sb")
                    nc.sync.dma_start(
                        emi_sb[:],
                        emi16.ap()[mp * 3 * 128:(mp + 1) * 3 * 128, :]
                        .rearrange("(l p) s -> p l s", p=128))
                    tls = ipool.tile([128, c.T], F32, tag="tls")
                    nc.sync.dma_start(
                        tls[:], tlocs.ap()[mp * 128:(mp + 1) * 128, :])
                    acc_ctx = tc.tile_pool(name=f"acc{mp}", bufs=1, space="PSUM")
                    apool = acc_ctx.__enter__()
                    accs = None
                    if not (c.skip_mm or c.skip_pe):
                        accs = [apool.tile([128, 128], F32, space=PSUM,
                                           name=f"acc{s}", tag=f"acc{s}")
                                for s in range(c.n_slc)]
                    ed_stub = None
                    if c.skip_gather:
                        ed_stub = keep.tile([128, 3, c.Tc, 128], BF16,
                                            name=f"eds{mp}", tag=f"eds{mp}")
                        nc.vector.memset(ed_stub[:], 0.25)
                    for ch in range(c.n_chunks):
                        if c.skip_gather:
                            ed = ed_stub
                        else:
                            ed = edpool.tile([128, 3, c.Tc, 128], BF16, tag="ed")
                            for l in range(3):
                                for (toff, nt, hi) in calls[ch][l]:
                                    src = (table_t[c.LO:c.n_nodes, :] if hi
                                           else table_t[0:c.LO, :])
                                    nc.gpsimd.dma_gather(
                                        out_ap=ed[:, l, toff:toff + nt, :],
                                        in_ap=src,
                                        idxs_ap=emi_sb[:, l,
                                                       (ch * c.Tc + toff) * 8:
                                                       (ch * c.Tc + toff + nt) * 8],
                                        num_idxs=nt * 128, num_idxs_reg=nt * 128,
                                        elem_size=128, single_packet=False)
                        rows = rowpool.tile([128, c.Tc, c.RW], BF16, tag="rows")
                        eftv = rows[:, :, 0:HID]
                        ed0 = ed[:, 0, :, 0:HID]
                        ed1 = ed[:, 1, :, 0:HID]
                        ed2 = ed[:, 2, :, 0:HID]
                        if c.skip_vec:
                            nc.vector.memset(rows[:], 0.125)
                        else:
                            nc.vector.tensor_tensor(eftv, ed0, ed2, ALU.add)
                            ta = mtpool.tile([128, c.Tc, HID], BF16, tag="ta")
                            c1bb = _ap_with(C1b[:], 0, [[0, c.Tc], [1, HID]])
                            nc.vector.tensor_tensor(ta[:], ed1, c1bb, ALU.mult)
                            tb = mtpool.tile([128, c.Tc, HID], BF16, tag="tb")
                            ed1s = _ap_with(ed1, 1, [list(ed1.ap[1]), [2, NPAIR], [-1, 2]])
                            c2bb = _ap_with(C2b[side][:], 0, [[0, c.Tc], [1, HID]])
                            nc.vector.tensor_tensor(tb[:], ed1s, c2bb, ALU.mult)
                            nc.vector.tensor_tensor(eftv, eftv, ta[:], ALU.add)
                            nc.vector.tensor_tensor(eftv, eftv, tb[:], ALU.add)
                            t5 = mtpool.tile([128, c.Tc, HID], BF16, tag="t5")
                            atb = _ap_with(ATTNb[mp][:], 0, [[0, c.Tc], [1, HID]])
                            nc.vector.tensor_tensor(t5[:], eftv, atb, ALU.mult)
                            ep = mtpool.tile([128, c.Tc, H], F32, tag="ep")
                            nc.vector.tensor_reduce(
                                ep[:], t5[:].rearrange("p t (h d) -> p t h d", d=D),
                                mybir.AxisListType.X, ALU.add)
                            epl = mtpool.tile([128, c.Tc, H], F32, tag="epl")
                            nc.vector.tensor_scalar_mul(epl[:], ep[:], 0.01)
                            nc.vector.tensor_tensor(epl[:], epl[:], ep[:], ALU.max)
                            av = rows[:, :, HID:HID + H]
                            nc.scalar.activation(av, epl[:], AF.Exp)
                            avb = _ap_with(rows[:], HID, [[c.RW, c.Tc], [1, H], [0, D]])
                            nc.vector.tensor_tensor(eftv, eftv, avb, ALU.mult)
                        if not c.skip_mm:
                            # one-hot [inst, tgt-in-slice] for all tiles of chunk
                            oh = ohpool.tile([128, c.Tc, 128], BF16, tag="oh")
                            iob = _ap_with(iotaF[:], 0, [[0, c.Tc], [1, 128]])
                            tlb = _ap_with(tls[:], ch * c.Tc,
                                           [[1, c.Tc], [0, 128]])
                            nc.vector.tensor_tensor(oh[:], iob, tlb, ALU.is_equal)
                            if not c.skip_pe:
                                for t in range(c.Tc):
                                    gt = ch * c.Tc + t
                                    s = slice_of[gt]
                                    nc.tensor.matmul(
                                        out=accs[s][:, 0:c.RW],
                                        lhsT=oh[:, t, :], rhs=rows[:, t, :],
                                        start=(gt == first[s]), stop=(gt == last[s]))
                    # drain accumulators: normalize + ELU
                    for s in range(c.n_slc):
                        red = hpool.tile([128, c.RW], F32, tag="red")
                        if c.skip_mm or c.skip_pe:
                            nc.vector.memset(red[:], 1.0)
                        else:
                            nc.vector.tensor_copy(red[:], accs[s][:, 0:c.RW])
                        den = hpool.tile([128, H], F32, tag="den")
                        nc.vector.tensor_scalar_add(den[:], red[:, HID:HID + H], 1e-9)
                        dinv = hpool.tile([128, H], F32, tag="dinv")
                        nc.vector.reciprocal(dinv[:], den[:])
                        ret = hpool.tile([128, HID], F32, tag="ret")
                        dinvb = _ap_with(dinv[:], 0, [[1, H], [0, D]])
                        nc.vector.tensor_tensor(ret[:], red[:, 0:HID], dinvb, ALU.mult)
                        neg = hpool.tile([128, HID], F32, tag="neg")
                        nc.vector.tensor_scalar_min(neg[:], ret[:], 0.0)
                        en = hpool.tile([128, HID], F32, tag="en")
                        nc.scalar.activation(en[:], neg[:], AF.Exp)
                        o = outs_all[:, mp, s, :]
                        nc.vector.tensor_scalar_max(ret[:], ret[:], 0.0)
                        nc.vector.tensor_scalar_add(en[:], en[:], -1.0)
                        nc.vector.tensor_tensor(o, ret[:], en[:], ALU.add)
                    acc_ctx.__exit__(None, None, None)

                # ---------- semantic attention + product MLP ----------
                if c.skip_semantic:
                    zot = keep.tile([128, 2], F32, tag="zot")
                    nc.vector.memset(zot[:], 0.5)
                    for s in range(c.n_slc):
                        nc.sync.dma_start(outd.ap()[s * 128:(s + 1) * 128, :], zot[:])
                sem_mps = [] if c.skip_semantic else list(range(c.n_mp))
                with tc.tile_pool(name="hd_ps", bufs=1, space="PSUM") as hpspool:
                    acc4 = keep.tile([1, c.n_mp], F32, tag="acc4")
                    nc.vector.memset(acc4[:], 0.0)
                    for mp in sem_mps:
                        w1sb = suw1sb if mp < 2 else siw1sb
                        b1rep = SUB1rep if mp < 2 else SIB1rep
                        w2rep = SUW2rep if mp < 2 else SIW2rep
                        for s in range(c.n_slc):
                            o = outs_all[:, mp, s, :]
                            oT_ps = hpspool.tile([HID, 128], F32, space=PSUM, tag="oT_ps", bufs=2)
                            nc.tensor.transpose(oT_ps[:], o, id128[:])
                            oT = hpool.tile([HID, 128], F32, tag="oT")
                            nc.vector.tensor_copy(oT[:], oT_ps[:])
                            tt = hpspool.tile([128, c.AV], F32, space=PSUM, tag="tt", bufs=2)
                            nc.tensor.matmul(out=tt[:], lhsT=oT[:], rhs=w1sb[:],
                                             start=True, stop=True)
                            th = hpool.tile([128, c.AV], F32, tag="th")
                            nc.vector.tensor_tensor(th[:], tt[:], b1rep[:], ALU.add)
                            nc.scalar.activation(th[:], th[:], AF.Tanh)
                            nc.vector.tensor_tensor(th[:], th[:], w2rep[:], ALU.mult)
                            rsum = hpool.tile([128, 1], F32, tag="rsum")
                            nc.vector.tensor_reduce(rsum[:], th[:],
                                                    mybir.AxisListType.X, ALU.add)
                            sp = hpspool.tile([1, 1], F32, space=PSUM, tag="sp")
                            nc.tensor.matmul(out=sp[:], lhsT=rsum[:], rhs=onescol[:],
                                             start=True, stop=True)
                            nc.vector.tensor_tensor(acc4[:, mp:mp + 1],
                                                    acc4[:, mp:mp + 1], sp[:], ALU.add)

                    sem_slcs = [] if c.skip_semantic else list(range(c.n_slc))
                    sin_t = dpool.tile([1, 128], F32, tag="sin")
                    sout_t = dpool.tile([1, 128], F32, tag="sout")
                    zrow = hpool.tile([1, 128], F32, tag="zrow")
                    nc.vector.memset(zrow[:], 0.0)
                    nc.sync.dma_start(sin_t[:], zrow[:])
                    if not c.skip_semantic:
                        nc.sync.dma_start(sin_t[0:1, 0:c.n_mp], acc4[:])
                    nc.gpsimd.collective_compute(
                        "AllReduce", ALU.add,
                        replica_groups=[list(range(c.n_cores))],
                        ins=[sin_t.opt()], outs=[sout_t.opt()],
                    )
                    s4 = hpool.tile([1, c.n_mp], F32, tag="s4")
                    nc.sync.dma_start(s4[:], sout_t[0:1, 0:c.n_mp])
                    e4 = hpool.tile([1, c.n_mp], F32, tag="e4")
                    nc.scalar.activation(e4[:], s4[:], AF.Exp, scale=1.0 / c.B)
                    beta = hpool.tile([1, c.n_mp], F32, tag="beta")
                    for sd in range(2):
                        ssum = hpool.tile([1, 1], F32, tag="ssum")
                        nc.vector.tensor_reduce(ssum[:], e4[:, 2 * sd:2 * sd + 2],
                                                mybir.AxisListType.X, ALU.add)
                        sinv = hpool.tile([1, 1], F32, tag="sinv")
                        nc.vector.reciprocal(sinv[:], ssum[:])
                        nc.vector.tensor_scalar_mul(beta[:, 2 * sd:2 * sd + 2],
                                                    e4[:, 2 * sd:2 * sd + 2], sinv[:])
                    bc_ps = hpspool.tile([128, c.n_mp], F32, space=PSUM, tag="bc_ps")
                    nc.tensor.matmul(out=bc_ps[:], lhsT=ones1[:], rhs=beta[:],
                                     start=True, stop=True)
                    bcol = keep.tile([128, c.n_mp], F32, tag="bcol")
                    nc.vector.tensor_copy(bcol[:], bc_ps[:])

                    for s in sem_slcs:
                        hu = hpool.tile([128, HID], F32, tag="hu")
                        hi_ = hpool.tile([128, HID], F32, tag="hi_")
                        t0 = hpool.tile([128, HID], F32, tag="t0")
                        nc.vector.tensor_scalar_mul(hu[:], outs_all[:, 0, s, :], bcol[:, 0:1])
                        nc.vector.tensor_scalar_mul(t0[:], outs_all[:, 1, s, :], bcol[:, 1:2])
                        nc.vector.tensor_tensor(hu[:], hu[:], t0[:], ALU.add)
                        nc.vector.tensor_scalar_mul(hi_[:], outs_all[:, 2, s, :], bcol[:, 2:3])
                        nc.vector.tensor_scalar_mul(t0[:], outs_all[:, 3, s, :], bcol[:, 3:4])
                        nc.vector.tensor_tensor(hi_[:], hi_[:], t0[:], ALU.add)
                        xx = hpool.tile([128, HID], F32, tag="xx")
                        nc.vector.tensor_tensor(xx[:], hu[:], hi_[:], ALU.mult)
                        xT_ps = hpspool.tile([HID, 128], F32, space=PSUM, tag="xT_ps")
                        nc.tensor.transpose(xT_ps[:], xx[:], id128[:])
                        xT = hpool.tile([HID, 128], F32, tag="xT")
                        nc.vector.tensor_copy(xT[:], xT_ps[:])
                        yy = hpspool.tile([128, c.CH], F32, space=PSUM, tag="yy")
                        nc.tensor.matmul(out=yy[:], lhsT=xT[:], rhs=cw1sb[:],
                                         start=True, stop=True)
                        ya = hpool.tile([128, c.CH], F32, tag="ya")
                        nc.vector.tensor_tensor(ya[:], yy[:], CB1rep[:], ALU.add)
                        nc.vector.tensor_scalar_max(ya[:], ya[:], 0.0)
                        l0t = hpool.tile([128, c.CH], F32, tag="l0t")
                        nc.vector.tensor_tensor(l0t[:], ya[:], CW20rep[:], ALU.mult)
                        l0 = hpool.tile([128, 1], F32, tag="l0")
                        nc.vector.tensor_reduce(l0[:], l0t[:], mybir.AxisListType.X, ALU.add)
                        nc.vector.tensor_tensor(l0t[:], ya[:], CW21rep[:], ALU.mult)
                        l1 = hpool.tile([128, 1], F32, tag="l1")
                        nc.vector.tensor_reduce(l1[:], l0t[:], mybir.AxisListType.X, ALU.add)
                        dl = hpool.tile([128, 1], F32, tag="dl")
                        ot = hpool.tile([128, 2], F32, tag="ot")
                        nc.vector.tensor_tensor(dl[:], l0[:], l1[:], ALU.subtract)
                        nc.scalar.activation(ot[:, 0:1], dl[:], AF.Sigmoid)
                        nc.vector.tensor_tensor(dl[:], l1[:], l0[:], ALU.subtract)
                        nc.scalar.activation(ot[:, 1:2], dl[:], AF.Sigmoid)
                        nc.sync.dma_start(outd.ap()[s * 128:(s + 1) * 128, :], ot[:])

    nc.compile()
    return nc


# ---------------------------------------------------------------------------
# host side: sharding / packing
# ---------------------------------------------------------------------------

def _mp_arrays(inputs, mp):
    if mp < 2:
        return np.asarray(inputs["emi_user"][mp]), np.asarray(inputs["tgt_user"][mp])
    return np.asarray(inputs["emi_item"][mp - 2]), np.asarray(inputs["tgt_item"][mp - 2])


def make_plan(inputs, cfg: Cfg):
    c = cfg
    tcs = np.zeros((c.n_mp, 8, 8), np.int64)
    for mp in range(c.n_mp):
        emi, tgt = _mp_arrays(inputs, mp)
        for k in range(c.n_cores):
            sel = (tgt >= k * c.B_loc) & (tgt < (k + 1) * c.B_loc)
            e, t = emi[sel], tgt[sel] - k * c.B_loc
            cls = ((e[:, 0] >= c.LO).astype(int) + 2 * (e[:, 1] >= c.LO) +
                   4 * (e[:, 2] >= c.LO))
            slc = t // 128
            cnt = np.zeros((8, 8), np.int64)
            np.add.at(cnt, (cls, slc), 1)
            tcs[mp] = np.maximum(tcs[mp], (cnt + 127) // 128)
        for s in range(8):
            if tcs[mp][:, s].sum() == 0:
                tcs[mp][7][s] = 1
    T_raw = int(tcs.sum(axis=(1, 2)).max())
    # choose Tc minimizing padded T, prefer larger chunks
    best = None
    cands = (c.force_Tc,) if c.force_Tc else (20, 22, 24, 26, 28, 30, 32, 36, 40)
    for Tc in cands:
        T_pad = ((T_raw + Tc - 1) // Tc) * Tc
        key = (T_pad, -Tc)
        if best is None or key < best[0]:
            best = (key, Tc, T_pad)
    _, Tc, T = best
    for mp in range(c.n_mp):
        tcs[mp][7][7] += T - tcs[mp].sum()
    return tcs, T, Tc


def _wrap16(vals):
    """[N] values (N % 16 == 0) -> [128, N/16] int16, q7 wrapped layout."""
    v = np.asarray(vals).astype(np.int16).reshape(-1, 16)
    return np.ascontiguousarray(np.tile(v.T, (8, 1)))


def _pack_metapath(emi, tgt, k, c: Cfg, tcs_mp):
    """Pack one (metapath, core) shard grouped by (class, slice).

    Returns (idx16 [3,128,T*8], tlocS [128, T] f32: slice-local target or
    -1e9 for padding)."""
    lo, hi = k * c.B_loc, (k + 1) * c.B_loc
    sel = np.nonzero((tgt >= lo) & (tgt < hi))[0]
    e_all, t_all = emi[sel], tgt[sel] - lo
    cls_all = ((e_all[:, 0] >= c.LO).astype(int) + 2 * (e_all[:, 1] >= c.LO) +
               4 * (e_all[:, 2] >= c.LO))
    slc_all = t_all // 128
    E = c.E_loc
    emi_sh = np.zeros((E, 3), np.int64)
    tlocS = np.full((E,), -1e9, np.float32)
    tpos = 0
    for cl in range(8):
        dummy = np.array([c.LO if (cl >> l) & 1 else 0 for l in range(3)], np.int64)
        for s in range(8):
            ntiles = int(tcs_mp[cl][s])
            if ntiles == 0:
                continue
            seg = np.nonzero((cls_all == cl) & (slc_all == s))[0]
            assert seg.size <= ntiles * 128
            base = tpos * 128
            emi_sh[base:base + seg.size] = e_all[seg]
            emi_sh[base + seg.size:base + ntiles * 128] = dummy
            tlocS[base:base + seg.size] = t_all[seg] - 128 * s
            tpos += ntiles
    assert tpos == c.T
    idx16 = []
    for l in range(3):
        v = emi_sh[:, l].copy()
        v[v >= c.LO] -= c.LO
        idx16.append(_wrap16(v))
    tl = np.ascontiguousarray(tlocS.reshape(c.T, 128).T)
    return np.stack(idx16), tl


def prepare(inputs, cfg: Cfg):
    """Plan and pack all shards. Returns in_maps (one per core)."""
    c = cfg
    tcs, T, Tc = make_plan(inputs, cfg)
    c.tiles_cs = tcs
    c.T = T
    c.Tc = Tc

    f0, f1 = np.asarray(inputs["feats0"]), np.asarray(inputs["feats1"])
    feats_all = np.concatenate([f0, f1], axis=0)
    attn4 = np.stack([np.asarray(inputs["attn_user"][p]).reshape(-1) for p in range(2)] +
                     [np.asarray(inputs["attn_item"][p]).reshape(-1) for p in range(2)])
    rv = np.asarray(inputs["r_vec"])[0].reshape(-1).astype(np.float32)

    in_maps = []
    for k in range(c.n_cores):
        m = {}
        lo_n = k * c.nodes_core
        fs = feats_all[lo_n:lo_n + c.nodes_core]
        pad = c.node_tiles * 128 - c.nodes_core
        if pad:
            fs = np.concatenate([fs, np.zeros((pad, c.F0), np.float32)], axis=0)
        m["feats"] = np.ascontiguousarray(fs, np.float32)
        tw = "0" if lo_n < f0.shape[0] else "1"
        for nm in ("pw", "pb", "w2", "b2", "g", "be"):
            m[nm] = np.asarray(inputs[f"tower{tw}_{nm}"], np.float32)
        m["rvec"] = rv
        m["attn"] = attn4.astype(np.float32)
        emi_l, tl_l = [], []
        for mp in range(c.n_mp):
            emi, tgt = _mp_arrays(inputs, mp)
            et, tl = _pack_metapath(emi, tgt, k, c, tcs[mp])
            emi_l.append(et)
            tl_l.append(tl)
        m["emi16"] = np.concatenate(emi_l).reshape(c.n_mp * 3 * 128, c.T * 8)
        m["tlocs"] = np.concatenate(tl_l).reshape(c.n_mp * 128, c.T)
        for nm in ("su_w1", "su_b1", "su_w2", "si_w1", "si_b1", "si_w2",
                   "cw1", "cb1", "cw2"):
            m[nm.replace("_", "")] = np.asarray(inputs[nm], np.float32)
        in_maps.append(m)
    return in_maps


# ---------------------------------------------------------------------------
# PJRT SPMD runner (axon path)
# ---------------------------------------------------------------------------


class SpmdRunner:
    def __init__(self, nc, n_cores: int):
        import jax
        from jax.sharding import Mesh, PartitionSpec, NamedSharding
        from jax.experimental.shard_map import shard_map
        from concourse.bass2jax import (
            _bass_exec_p, install_neuronx_cc_hook, partition_id_tensor)

        self.jax = jax
        install_neuronx_cc_hook()
        self.nc = nc
        self.n_cores = n_cores
        partition_name = nc.partition_id_tensor.name if nc.partition_id_tensor else None
        in_names, out_names, out_avals, zero_outs = [], [], [], []
        for alloc in nc.m.functions[0].allocations:
            if not isinstance(alloc, mybir.MemoryLocationSet):
                continue
            name = alloc.memorylocations[0].name
            if alloc.kind == "ExternalInput":
                if name != partition_name:
                    in_names.append(name)
            elif alloc.kind == "ExternalOutput":
                out_names.append(name)
                shape = tuple(alloc.tensor_shape)
                dtype = mybir.dt.np(alloc.dtype)
                out_avals.append(jax.core.ShapedArray(shape, dtype))
                zero_outs.append(np.zeros(shape, dtype))
        self.dbg_name = nc.dbg_addr.name if nc.dbg_addr is not None else None
        n_params = len(in_names)
        in_names = in_names + out_names
        if partition_name is not None:
            in_names.append(partition_name)
        self.in_names, self.out_names = in_names, out_names
        self.n_params, self.out_avals, self.zero_outs = n_params, out_avals, zero_outs

        def _body(*args):
            operands = list(args)
            if partition_name is not None:
                operands.append(partition_id_tensor())
            outs = _bass_exec_p.bind(
                *operands,
                out_avals=tuple(out_avals),
                in_names=tuple(in_names),
                out_names=tuple(out_names),
                lowering_input_output_aliases=(),
                sim_require_finite=True,
                sim_require_nnan=True,
                nc=nc,
            )
            return tuple(outs)

        devices = jax.devices()[:n_cores]
        assert len(devices) == n_cores
        self.mesh = Mesh(np.asarray(devices), ("core",))
        donate = tuple(range(n_params, n_params + len(out_names)))
        in_specs = (PartitionSpec("core"),) * (n_params + len(out_names))
        out_specs = (PartitionSpec("core"),) * len(out_names)
        self.sharded = jax.jit(
            shard_map(_body, mesh=self.mesh, in_specs=in_specs,
                      out_specs=out_specs, check_rep=False),
            donate_argnums=donate, keep_unused=True)
        self.sharding = NamedSharding(self.mesh, PartitionSpec("core"))

    def stage_inputs(self, in_maps):
        jax = self.jax
        if self.dbg_name is not None:
            in_maps = [{**m, self.dbg_name: np.zeros((1, 2), np.uint32)}
                       for m in in_maps]
        staged = []
        for i in range(self.n_params):
            name = self.in_names[i]
            arr = np.concatenate([np.asarray(m[name]) for m in in_maps], axis=0)
            staged.append(jax.device_put(arr, self.sharding))
        jax.block_until_ready(staged)
        self.staged = staged

    def _zeros(self):
        jax = self.jax
        zs = [jax.device_put(
            np.zeros((self.n_cores * z.shape[0], *z.shape[1:]), z.dtype),
            self.sharding) for z in self.zero_outs]
        jax.block_until_ready(zs)
        return zs

    def run(self):
        jax = self.jax
        outs = self.sharded(*self.staged, *self._zeros())
        jax.block_until_ready(outs)
        return [
            {name: np.asarray(outs[i]).reshape(self.n_cores, *self.out_avals[i].shape)[k]
             for i, name in enumerate(self.out_names)}
            for k in range(self.n_cores)
        ]

    def bench(self, iters=20, warmup=3):
        import time
        jax = self.jax
        times = []
        for it in range(warmup + iters):
            zs = self._zeros()
            t0 = time.perf_counter()
            outs = self.sharded(*self.staged, *zs)
            jax.block_until_ready(outs)
            dt = time.perf_counter() - t0
            if it >= warmup:
                times.append(dt)
            del outs
        times = np.array(times)
        return {"min_s": float(times.min()), "med_s": float(np.median(times)),
                "mean_s": float(times.mean()), "n": iters}


_CACHE = {}


def kernel(**inputs) -> np.ndarray:
    cfg = Cfg()
    in_maps = prepare(inputs, cfg)
    key = (cfg.T, cfg.Tc, cfg.tiles_cs.tobytes())
    if key not in _CACHE:
        nc = build_program(cfg)
        _CACHE[key] = (nc, SpmdRunner(nc, cfg.n_cores))
    nc, runner = _CACHE[key]
    runner.stage_inputs(in_maps)
    res = runner.run()
    out = np.empty((cfg.B, 2), np.float32)
    for k in range(cfg.n_cores):
        out[k * cfg.B_loc:(k + 1) * cfg.B_loc] = res[k]["out"]
    return out


# revision 11
# speedup vs baseline: 1.7620x; 1.7620x over previous
"""Trainium2 Bass kernel for MAGNN link prediction (nn_MAGNN_lp) — v2.

Differences from v1 (dump + regather): the segment softmax/sum is computed
with one-hot matmuls into PSUM accumulators, eliminating the 13MB-per-
metapath row dump to DRAM and the per-target regather entirely.

Sharding: B=8192 targets across 8 cores (1024 each); instances sharded by
target range so aggregation is core-local. Node towers sharded by rows
(5000/core), projected node table ([40000, 128] bf16, vec + zero pad per
row) AllGathered in DRAM. Instances are packed into 128-instance tiles
grouped by (lo/hi gather class, target slice of 128); each tile is
slice-pure, so its contributions accumulate into one of 8 PSUM banks via
matmul(onehot[inst,tgt]^T @ [w*eft | w]). The one-hot is built on DVE by
comparing an iota row against host-shifted local target ids. After each
metapath: normalize + ELU into SBUF; then semantic attention (AllReduce of
4 scalars), product MLP, 2-way softmax.
"""
import math
from dataclasses import dataclass

import ml_dtypes
import numpy as np

import concourse.bass as bass
import concourse.mybir as mybir
import concourse.tile as tile
from concourse import bacc
from concourse.masks import make_identity

F32 = mybir.dt.float32
BF16 = mybir.dt.bfloat16
I32 = mybir.dt.int32
I16 = mybir.dt.int16
AF = mybir.ActivationFunctionType
ALU = mybir.AluOpType
PSUM = "PSUM"


@dataclass
class Cfg:
    n_cores: int = 8
    B: int = 8192
    HID: int = 64
    H: int = 8
    D: int = 8
    F0: int = 512
    AV: int = 128
    CH: int = 128
    n_nodes: int = 40000
    LO: int = 32768        # lo/hi table split (int16 gather index limit)
    T: int = 260           # 128-instance tiles per metapath per core
    Tc: int = 20           # tiles per processing chunk
    n_mp: int = 4
    RW: int = 72           # row width: 64 weighted values + 8 exp weights
    gelu: bool = True      # False: Tanh stand-in (CoreSim lacks Gelu)
    shared_table: bool = True
    tiles_cs: np.ndarray | None = None   # [n_mp, 8 cls, 8 slc]
    skip_gather: bool = False
    skip_mm: bool = False
    skip_pe: bool = False
    skip_vec: bool = False
    skip_semantic: bool = False
    skip_allgather: bool = False
    skip_tower: bool = False
    force_Tc: int = 20

    @property
    def B_loc(self):
        return self.B // self.n_cores

    @property
    def nodes_core(self):
        return self.n_nodes // self.n_cores

    @property
    def node_tiles(self):
        return (self.nodes_core + 127) // 128

    @property
    def E_loc(self):
        return self.T * 128

    @property
    def n_chunks(self):
        return self.T // self.Tc

    @property
    def kF(self):
        return self.F0 // 128

    @property
    def n_slc(self):
        return self.B_loc // 128


def _ap_with(ap, offset_delta, tail_dims):
    """Copy an AP, keeping its partition dim, replacing trailing free dims."""
    return bass.AP(ap.tensor, ap.offset + offset_delta,
                   [list(ap.ap[0])] + [list(d) for d in tail_dims])


def _class_segments(tiles_per_class):
    segs, t = [], 0
    for cls in range(8):
        n = int(tiles_per_class[cls])
        if n:
            segs.append((cls, t, t + n))
            t += n
    return segs


def _gather_calls(tiles_per_class, Tc, n_chunks):
    """calls[ch][l] = [(toff_rel, ntiles, hi)], adjacent same-hi merged."""
    segs = _class_segments(tiles_per_class)
    calls = []
    for ch in range(n_chunks):
        c0, c1 = ch * Tc, (ch + 1) * Tc
        per_l = []
        for l in range(3):
            lst = []
            for cls, s0, s1 in segs:
                a, b = max(c0, s0), min(c1, s1)
                if a >= b:
                    continue
                hi = bool((cls >> l) & 1)
                if lst and lst[-1][2] == hi and lst[-1][0] + lst[-1][1] == a - c0:
                    lst[-1] = (lst[-1][0], lst[-1][1] + (b - a), hi)
                else:
                    lst.append((a - c0, b - a, hi))
            per_l.append(lst)
        calls.append(per_l)
    return calls


def _slice_map(tcs_mp):
    """tile index -> slice, plus first/last tile per slice."""
    slice_of = []
    for cl in range(8):
        for s in range(8):
            slice_of += [s] * int(tcs_mp[cl][s])
    first, last = {}, {}
    for t, s in enumerate(slice_of):
        if s not in first:
            first[s] = t
        last[s] = t
    return slice_of, first, last


def _finish(nc):
    nc.compile()
    return nc


def build_program(cfg: Cfg):
    c = cfg
    assert c.tiles_cs is not None
    nc = bacc.Bacc("TRN2", target_bir_lowering=False, debug=False,
                   num_devices=c.n_cores)

    def di(name, shape, dtype=F32):
        return nc.dram_tensor(name, list(shape), dtype, kind="ExternalInput")

    T8 = c.T * 8
    featsT = di("featsT", (c.F0, c.node_tiles * 128))
    pw = di("pw", (c.F0, c.HID))
    pb = di("pb", (c.HID,))
    w2 = di("w2", (c.HID, c.HID))
    b2 = di("b2", (c.HID,))
    g = di("g", (c.HID,))
    be = di("be", (c.HID,))
    rvec = di("rvec", (c.HID,))
    attn = di("attn", (c.n_mp, c.HID))
    emi16 = di("emi16", (c.n_mp * 3 * 128, T8), I16)
    tlocs = di("tlocs", (c.n_mp * 128, c.T), BF16)
    suw1 = di("suw1", (c.HID, c.AV))
    sub1 = di("sub1", (c.AV,))
    suw2 = di("suw2", (c.AV,))
    siw1 = di("siw1", (c.HID, c.AV))
    sib1 = di("sib1", (c.AV,))
    siw2 = di("siw2", (c.AV,))
    cw1 = di("cw1", (c.HID, c.CH))
    cb1 = di("cb1", (c.CH,))
    cw2 = di("cw2", (c.CH, 2))
    outd = nc.dram_tensor("out", [c.B_loc, 2], F32, kind="ExternalOutput")

    HID, H, D = c.HID, c.H, c.D
    NPAIR = HID // 2

    with tile.TileContext(nc) as tc:
        with (
            tc.tile_pool(name="const", bufs=1) as kpool,
            tc.tile_pool(name="dram", bufs=1, space="DRAM") as dpool,
        ):
            pk_ctx = tc.tile_pool(name="ps_const", bufs=1, space="PSUM")
            pkpool = pk_ctx.__enter__()
            # ---------- constants ----------
            id128 = kpool.tile([128, 128], F32, tag="id128")
            make_identity(nc, id128[:])
            ones1 = kpool.tile([1, 128], F32, tag="ones1")
            nc.vector.memset(ones1[:], 1.0)
            onescol = kpool.tile([128, 1], F32, tag="onescol")
            nc.vector.memset(onescol[:], 1.0)
            epscol = kpool.tile([128, 1], F32, tag="epscol")
            nc.vector.memset(epscol[:], 1e-5)
            iotaI = kpool.tile([128, 128], I32, tag="iotaI")
            nc.gpsimd.iota(iotaI[:], pattern=[[1, 128]], base=0,
                           channel_multiplier=0)
            iotaF = kpool.tile([128, 128], F32, tag="iotaF")
            nc.vector.tensor_copy(iotaF[:], iotaI[:])
            iotaB = kpool.tile([128, 128], BF16, tag="iotaB")
            nc.vector.tensor_copy(iotaB[:], iotaI[:])

            def rep_row(dram_vec, n, scale=None, tag=None):
                row = kpool.tile([1, n], F32, tag=f"{tag}_row")
                nc.sync.dma_start(row[:], dram_vec)
                ps = pkpool.tile([128, 512], F32, space=PSUM, tag="reppsum")
                nc.tensor.matmul(out=ps[:, :n], lhsT=ones1[:], rhs=row[:],
                                 start=True, stop=True)
                rep = kpool.tile([128, n], F32, tag=tag)
                if scale is None:
                    nc.vector.tensor_copy(rep[:], ps[:, :n])
                else:
                    nc.vector.tensor_scalar_mul(rep[:], ps[:, :n], scale)
                return rep

            def vrow(x):
                return x.ap().rearrange("(o a) -> o a", o=1)

            PBrep = rep_row(vrow(pb), HID, tag="PBrep")
            B2rep = rep_row(vrow(b2), HID, tag="B2rep")
            G3rep = rep_row(vrow(g), HID, scale=1.0 / 3.0, tag="G3rep")
            BE3rep = rep_row(vrow(be), HID, scale=1.0 / 3.0, tag="BE3rep")
            SUB1rep = rep_row(vrow(sub1), c.AV, tag="SUB1rep")
            SIB1rep = rep_row(vrow(sib1), c.AV, tag="SIB1rep")
            SUW2rep = rep_row(vrow(suw2), c.AV, tag="SUW2rep")
            SIW2rep = rep_row(vrow(siw2), c.AV, tag="SIW2rep")
            CB1rep = rep_row(vrow(cb1), c.CH, tag="CB1rep")
            CW20rep = rep_row(cw2.ap()[:, 0:1].rearrange("a o -> o a"), c.CH, tag="CW20rep")
            CW21rep = rep_row(cw2.ap()[:, 1:2].rearrange("a o -> o a"), c.CH, tag="CW21rep")
            ATTNrep = [rep_row(attn.ap()[mp:mp + 1, :], HID, tag=f"ATTN{mp}")
                       for mp in range(c.n_mp)]

            # ---------- rotation constants (normalize r0 on device) ----------
            rcol = kpool.tile([HID, 1], F32, tag="rcol")
            nc.sync.dma_start(rcol[:], rvec.ap().rearrange("(p o) -> p o", o=1))
            idh = kpool.tile([HID, HID], F32, tag="idh")
            make_identity(nc, idh[:])
            Sp = kpool.tile([HID, HID], F32, tag="Sp")
            nc.vector.memset(Sp[:], 0.0)
            nc.vector.tensor_copy(Sp[:, 1:HID], idh[:, 0:HID - 1])
            Sm = kpool.tile([HID, HID], F32, tag="Sm")
            nc.vector.memset(Sm[:], 0.0)
            nc.vector.tensor_copy(Sm[:, 0:HID - 1], idh[:, 1:HID])
            pidx = kpool.tile([HID, 1], I32, tag="pidx")
            nc.gpsimd.iota(pidx[:], pattern=[[0, 1]], base=0, channel_multiplier=1)
            podd_i = kpool.tile([HID, 1], I32, tag="podd_i")
            nc.vector.tensor_scalar(podd_i[:], pidx[:], 1, None, ALU.bitwise_and)
            podd = kpool.tile([HID, 1], F32, tag="podd")
            nc.vector.tensor_copy(podd[:], podd_i[:])
            peven = kpool.tile([HID, 1], F32, tag="peven")
            nc.vector.tensor_scalar(peven[:], podd[:], -1.0, -1.0, ALU.add, ALU.mult)
            Spe = kpool.tile([HID, HID], F32, tag="Spe")
            nc.vector.tensor_scalar_mul(Spe[:], Sp[:], peven[:])
            Smo = kpool.tile([HID, HID], F32, tag="Smo")
            nc.vector.tensor_scalar_mul(Smo[:], Sm[:], podd[:])
            Ie = kpool.tile([HID, HID], F32, tag="Ie")
            nc.vector.tensor_scalar_mul(Ie[:], idh[:], peven[:])
            Io = kpool.tile([HID, HID], F32, tag="Io")
            nc.vector.tensor_scalar_mul(Io[:], idh[:], podd[:])
            M2 = kpool.tile([HID, HID], F32, tag="M2")
            nc.vector.tensor_tensor(M2[:], idh[:], Spe[:], ALU.add)
            nc.vector.tensor_tensor(M2[:], M2[:], Smo[:], ALU.add)
            Me = kpool.tile([HID, HID], F32, tag="Me")
            nc.vector.tensor_tensor(Me[:], Ie[:], Spe[:], ALU.add)
            Mo = kpool.tile([HID, HID], F32, tag="Mo")
            nc.vector.tensor_tensor(Mo[:], Io[:], Smo[:], ALU.add)
            sqc = kpool.tile([HID, 1], F32, tag="sqc")
            nc.vector.tensor_tensor(sqc[:], rcol[:], rcol[:], ALU.mult)
            n2 = pkpool.tile([HID, 1], F32, space=PSUM, tag="n2")
            nc.tensor.matmul(out=n2[:], lhsT=M2[:], rhs=sqc[:], start=True, stop=True)
            nrm = kpool.tile([HID, 1], F32, tag="nrm")
            nc.scalar.activation(nrm[:], n2[:], AF.Sqrt)
            invn = kpool.tile([HID, 1], F32, tag="invn")
            nc.vector.reciprocal(invn[:], nrm[:])
            rn = kpool.tile([HID, 1], F32, tag="rn")
            nc.vector.tensor_scalar_mul(rn[:], rcol[:], invn[:])
            cr2 = pkpool.tile([HID, 1], F32, space=PSUM, tag="cr2")
            nc.tensor.matmul(out=cr2[:], lhsT=Me[:], rhs=rn[:], start=True, stop=True)
            ci2 = pkpool.tile([HID, 1], F32, space=PSUM, tag="ci2")
            nc.tensor.matmul(out=ci2[:], lhsT=Mo[:], rhs=rn[:], start=True, stop=True)
            cr2s = kpool.tile([HID, 1], F32, tag="cr2s")
            nc.vector.tensor_copy(cr2s[:], cr2[:])
            ci2s = kpool.tile([HID, 1], F32, tag="ci2s")
            nc.vector.tensor_copy(ci2s[:], ci2[:])
            crrow_ps = pkpool.tile([1, HID], F32, space=PSUM, tag="crrow_ps")
            nc.tensor.matmul(out=crrow_ps[:], lhsT=cr2s[:], rhs=idh[:], start=True, stop=True)
            crrow = kpool.tile([1, HID], F32, tag="crrow")
            nc.vector.tensor_copy(crrow[:], crrow_ps[:])
            cirow_ps = pkpool.tile([1, HID], F32, space=PSUM, tag="cirow_ps")
            nc.tensor.matmul(out=cirow_ps[:], lhsT=ci2s[:], rhs=idh[:], start=True, stop=True)
            cirow = kpool.tile([1, HID], F32, tag="cirow")
            nc.vector.tensor_copy(cirow[:], cirow_ps[:])
            fidx = kpool.tile([1, HID], I32, tag="fidx")
            nc.gpsimd.iota(fidx[:], pattern=[[1, HID]], base=0, channel_multiplier=0)
            fodd_i = kpool.tile([1, HID], I32, tag="fodd_i")
            nc.vector.tensor_scalar(fodd_i[:], fidx[:], 1, None, ALU.bitwise_and)
            fsign = kpool.tile([1, HID], F32, tag="fsign")
            nc.vector.tensor_copy(fsign[:], fodd_i[:])
            nc.vector.tensor_scalar(fsign[:], fsign[:], -2.0, 1.0, ALU.mult, ALU.add)
            c2urow = kpool.tile([1, HID], F32, tag="c2urow")
            nc.vector.tensor_tensor(c2urow[:], cirow[:], fsign[:], ALU.mult)
            c2irow = kpool.tile([1, HID], F32, tag="c2irow")
            nc.vector.tensor_scalar_mul(c2irow[:], c2urow[:], -1.0)

            def rep_from_row(row, n, tag, dtype=F32):
                ps = pkpool.tile([128, 512], F32, space=PSUM, tag="reppsum")
                nc.tensor.matmul(out=ps[:, :n], lhsT=ones1[:], rhs=row[:],
                                 start=True, stop=True)
                rep = kpool.tile([128, n], dtype, tag=tag)
                nc.vector.tensor_copy(rep[:], ps[:, :n])
                return rep

            C1b = rep_from_row(crrow, HID, "C1b", BF16)
            C2b = [rep_from_row(c2urow, HID, "C2ub", BF16),
                   rep_from_row(c2irow, HID, "C2ib", BF16)]
            ATTNb = []
            for mp in range(c.n_mp):
                ab = kpool.tile([128, HID], BF16, tag=f"ATTNb{mp}")
                nc.vector.tensor_copy(ab[:], ATTNrep[mp][:])
                ATTNb.append(ab)

            pwsb = kpool.tile([128, c.kF, HID], F32, tag="pwsb")
            nc.sync.dma_start(pwsb[:], pw.ap().rearrange("(a p) c -> p a c", p=128))
            w2sb = kpool.tile([HID, HID], F32, tag="w2sb")
            nc.sync.dma_start(w2sb[:], w2.ap())
            suw1sb = kpool.tile([HID, c.AV], F32, tag="suw1sb")
            nc.sync.dma_start(suw1sb[:], suw1.ap())
            siw1sb = kpool.tile([HID, c.AV], F32, tag="siw1sb")
            nc.sync.dma_start(siw1sb[:], siw1.ap())
            cw1sb = kpool.tile([HID, c.CH], F32, tag="cw1sb")
            nc.sync.dma_start(cw1sb[:], cw1.ap())

            pk_ctx.__exit__(None, None, None)

            # ---------- tower (bf16 table rows: [vec(64) | zeros(64)]) ----------
            tower_t = dpool.tile([c.nodes_core, 128], BF16, tag="tower")
            table_t = dpool.tile(
                [c.n_nodes, 128], BF16, tag="table",
                addr_space="Shared" if c.shared_table else "Local")
            with (
                tc.tile_pool(name="tw_x", bufs=2) as xpool,
                tc.tile_pool(name="tw_ps", bufs=1, space="PSUM") as tpspool,
                tc.tile_pool(name="tw_s", bufs=3) as tspool,
                tc.tile_pool(name="tw_keep", bufs=1) as tkeep,
            ):
                if c.skip_tower:
                    zb16 = tkeep.tile([128, 128], BF16, tag="zb16")
                    nc.vector.memset(zb16[:], 0.5)
                    for j in range(c.node_tiles):
                        rows_n = min(128, c.nodes_core - j * 128)
                        nc.sync.dma_start(tower_t[j * 128:j * 128 + rows_n, :],
                                          zb16[:rows_n, :])
                # pass 1: all-Gelu; buffer centered activations + variances
                if not c.skip_tower:
                    ycs = tkeep.tile([128, c.node_tiles, HID], F32, tag="ycs")
                    vvs = tkeep.tile([128, c.node_tiles], F32, tag="vvs")
                NTOT = c.node_tiles * 128
                for j in range(c.node_tiles if not c.skip_tower else 0):
                    xT = xpool.tile([128, c.kF, 128], F32, tag="xT")
                    nc.sync.dma_start(
                        xT[:], bass.AP(featsT.ap().tensor, j * 128,
                                       [[NTOT, 128], [128 * NTOT, c.kF], [1, 128]]))
                    z = tpspool.tile([128, HID], F32, space=PSUM, tag="z")
                    for kk in range(c.kF):
                        nc.tensor.matmul(out=z[:], lhsT=xT[:, kk, :], rhs=pwsb[:, kk, :],
                                         start=(kk == 0), stop=(kk == c.kF - 1))
                    zb = tspool.tile([128, HID], F32, tag="zb")
                    nc.vector.tensor_tensor(zb[:], z[:], PBrep[:], ALU.add)
                    h = tspool.tile([128, HID], F32, tag="h")
                    nc.scalar.activation(h[:], zb[:], AF.Gelu if c.gelu else AF.Tanh)
                    hT_ps = tpspool.tile([HID, 128], F32, space=PSUM, tag="hT_ps")
                    nc.tensor.transpose(hT_ps[:], h[:], id128[:])
                    hT = tspool.tile([HID, 128], F32, tag="hT")
                    nc.vector.tensor_copy(hT[:], hT_ps[:])
                    y = tpspool.tile([128, HID], F32, space=PSUM, tag="y")
                    nc.tensor.matmul(out=y[:], lhsT=hT[:], rhs=w2sb[:], start=True, stop=True)
                    ys = tspool.tile([128, HID], F32, tag="ys")
                    nc.vector.tensor_tensor(ys[:], y[:], B2rep[:], ALU.add)
                    nc.vector.tensor_tensor(ys[:], ys[:], zb[:], ALU.add)
                    mu = tspool.tile([128, 1], F32, tag="mu")
                    nc.vector.tensor_reduce(mu[:], ys[:], mybir.AxisListType.X, ALU.add)
                    nc.vector.tensor_scalar_mul(mu[:], mu[:], 1.0 / HID)
                    yc = ycs[:, j, :]
                    nc.vector.tensor_scalar(yc, ys[:], mu[:], None, ALU.subtract)
                    sq = tspool.tile([128, HID], F32, tag="sq")
                    nc.vector.tensor_tensor(sq[:], yc, yc, ALU.mult)
                    nc.vector.tensor_reduce(vvs[:, j:j + 1], sq[:],
                                            mybir.AxisListType.X, ALU.add)
                # pass 2: one Sqrt for all tiles, then scale + write
                if not c.skip_tower:
                    sdv = tkeep.tile([128, c.node_tiles], F32, tag="sdv")
                    nc.scalar.activation(sdv[:], vvs[:], AF.Sqrt, bias=epscol[:],
                                         scale=1.0 / HID)
                    invs = tkeep.tile([128, c.node_tiles], F32, tag="invs")
                    nc.vector.reciprocal(invs[:], sdv[:])
                for j in range(c.node_tiles if not c.skip_tower else 0):
                    tbl = tspool.tile([128, 128], BF16, tag="tbl")
                    nc.vector.memset(tbl[:, HID:128], 0.0)
                    tn = tspool.tile([128, HID], F32, tag="tn")
                    nc.vector.tensor_scalar_mul(tn[:], ycs[:, j, :], invs[:, j:j + 1])
                    nc.vector.tensor_tensor(tn[:], tn[:], G3rep[:], ALU.mult)
                    nc.vector.tensor_tensor(tbl[:, 0:HID], tn[:], BE3rep[:], ALU.add)
                    rows_n = min(128, c.nodes_core - j * 128)
                    nc.sync.dma_start(tower_t[j * 128:j * 128 + rows_n, :], tbl[:rows_n, :])

            if not c.skip_allgather:
                nc.gpsimd.collective_compute(
                    "AllGather", ALU.bypass,
                    replica_groups=[list(range(c.n_cores))],
                    ins=[tower_t.opt()], outs=[table_t.opt()],
                )

            # ---------- metapaths: gather, rotate, logits, onehot-matmul ----------
            outs_all = None
            with (
                tc.tile_pool(name="mp_idx", bufs=2) as ipool,
                tc.tile_pool(name="mp_ed", bufs=2) as edpool,
                tc.tile_pool(name="mp_row", bufs=2) as rowpool,
                tc.tile_pool(name="mp_oh", bufs=2) as ohpool,
                tc.tile_pool(name="mp_tmp", bufs=2) as mtpool,
                tc.tile_pool(name="mp_keep", bufs=1) as keep,
                tc.tile_pool(name="mp_hs", bufs=3) as hpool,
            ):
                outs_all = keep.tile([128, c.n_mp, c.n_slc, HID], F32, tag="outs_all")
                for mp in range(c.n_mp):
                    side = 0 if mp < 2 else 1
                    tcs_mp = c.tiles_cs[mp]
                    calls = _gather_calls(tcs_mp.sum(axis=1), c.Tc, c.n_chunks)
                    slice_of, first, last = _slice_map(tcs_mp)
                    emi_sb = ipool.tile([128, 3, T8], I16, tag="emi_sb")
                    nc.sync.dma_start(
                        emi_sb[:],
                        emi16.ap()[mp * 3 * 128:(mp + 1) * 3 * 128, :]
                        .rearrange("(l p) s -> p l s", p=128))
                    tls = ipool.tile([128, c.T], BF16, tag="tls")
                    nc.sync.dma_start(
                        tls[:], tlocs.ap()[mp * 128:(mp + 1) * 128, :])
                    acc_ctx = tc.tile_pool(name=f"acc{mp}", bufs=1, space="PSUM")
                    apool = acc_ctx.__enter__()
                    accs = None
                    if not (c.skip_mm or c.skip_pe):
                        accs = [apool.tile([128, 128], F32, space=PSUM,
                                           name=f"acc{s}", tag=f"acc{s}")
                                for s in range(c.n_slc)]
                    ed_stub = None
                    if c.skip_gather:
                        ed_stub = keep.tile([128, 3, c.Tc, 128], BF16,
                                            name=f"eds{mp}", tag=f"eds{mp}")
                        nc.vector.memset(ed_stub[:], 0.25)
                    for ch in range(c.n_chunks):
                        if c.skip_gather:
                            ed = ed_stub
                        else:
                            ed = edpool.tile([128, 3, c.Tc, 128], BF16, tag="ed")
                            for l in range(3):
                                for (toff, nt, hi) in calls[ch][l]:
                                    src = (table_t[c.LO:c.n_nodes, :] if hi
                                           else table_t[0:c.LO, :])
                                    nc.gpsimd.dma_gather(
                                        out_ap=ed[:, l, toff:toff + nt, :],
                                        in_ap=src,
                                        idxs_ap=emi_sb[:, l,
                                                       (ch * c.Tc + toff) * 8:
                                                       (ch * c.Tc + toff + nt) * 8],
                                        num_idxs=nt * 128, num_idxs_reg=nt * 128,
                                        elem_size=128, single_packet=False)
                        rows = rowpool.tile([128, c.Tc, c.RW], BF16, tag="rows")
                        eftv = rows[:, :, 0:HID]
                        ed0 = ed[:, 0, :, 0:HID]
                        ed1 = ed[:, 1, :, 0:HID]
                        ed2 = ed[:, 2, :, 0:HID]
                        if c.skip_vec:
                            nc.vector.memset(rows[:], 0.125)
                        else:
                            nc.vector.tensor_tensor(eftv, ed0, ed2, ALU.add)
                            ta = mtpool.tile([128, c.Tc, HID], BF16, tag="ta")
                            c1bb = _ap_with(C1b[:], 0, [[0, c.Tc], [1, HID]])
                            nc.vector.tensor_tensor(ta[:], ed1, c1bb, ALU.mult)
                            tb = mtpool.tile([128, c.Tc, HID], BF16, tag="tb")
                            ed1s = _ap_with(ed1, 1, [list(ed1.ap[1]), [2, NPAIR], [-1, 2]])
                            c2bb = _ap_with(C2b[side][:], 0, [[0, c.Tc], [1, HID]])
                            nc.vector.tensor_tensor(tb[:], ed1s, c2bb, ALU.mult)
                            nc.vector.tensor_tensor(eftv, eftv, ta[:], ALU.add)
                            nc.vector.tensor_tensor(eftv, eftv, tb[:], ALU.add)
                            t5 = mtpool.tile([128, c.Tc, HID], BF16, tag="t5")
                            atb = _ap_with(ATTNb[mp][:], 0, [[0, c.Tc], [1, HID]])
                            nc.vector.tensor_tensor(t5[:], eftv, atb, ALU.mult)
                            ep = mtpool.tile([128, c.Tc, H], F32, tag="ep")
                            nc.vector.tensor_reduce(
                                ep[:], t5[:].rearrange("p t (h d) -> p t h d", d=D),
                                mybir.AxisListType.X, ALU.add)
                            epl = mtpool.tile([128, c.Tc, H], F32, tag="epl")
                            nc.vector.tensor_scalar_mul(epl[:], ep[:], 0.01)
                            nc.vector.tensor_tensor(epl[:], epl[:], ep[:], ALU.max)
                            av = rows[:, :, HID:HID + H]
                            nc.scalar.activation(av, epl[:], AF.Exp)
                            avb = _ap_with(rows[:], HID, [[c.RW, c.Tc], [1, H], [0, D]])
                            nc.vector.tensor_tensor(eftv, eftv, avb, ALU.mult)
                        if not c.skip_mm:
                            # one-hot [inst, tgt-in-slice] for all tiles of chunk
                            oh = ohpool.tile([128, c.Tc, 128], BF16, tag="oh")
                            iob = _ap_with(iotaB[:], 0, [[0, c.Tc], [1, 128]])
                            tlb = _ap_with(tls[:], ch * c.Tc,
                                           [[1, c.Tc], [0, 128]])
                            nc.vector.tensor_tensor(oh[:], iob, tlb, ALU.is_equal)
                            if not c.skip_pe:
                                for t in range(c.Tc):
                                    gt = ch * c.Tc + t
                                    s = slice_of[gt]
                                    nc.tensor.matmul(
                                        out=accs[s][:, 0:c.RW],
                                        lhsT=oh[:, t, :], rhs=rows[:, t, :],
                                        start=(gt == first[s]), stop=(gt == last[s]))
                    # drain accumulators: batched normalize + ELU
                    NS = c.n_slc
                    red = hpool.tile([128, NS, 128], F32, tag="red")
                    if c.skip_mm or c.skip_pe:
                        nc.vector.memset(red[:], 1.0)
                    else:
                        for s_ in range(NS):
                            nc.vector.tensor_copy(red[:, s_, 0:c.RW],
                                                  accs[s_][:, 0:c.RW])
                    den = hpool.tile([128, NS, H], F32, tag="den")
                    nc.vector.tensor_scalar_add(den[:], red[:, :, HID:HID + H], 1e-9)
                    dinv = hpool.tile([128, NS, H], F32, tag="dinv")
                    nc.vector.reciprocal(dinv[:], den[:])
                    ret = hpool.tile([128, NS, HID], F32, tag="ret")
                    dinvb = _ap_with(dinv[:], 0, [[H, NS], [1, H], [0, D]])
                    nc.vector.tensor_tensor(ret[:], red[:, :, 0:HID], dinvb, ALU.mult)
                    neg = hpool.tile([128, NS, HID], F32, tag="neg")
                    nc.vector.tensor_scalar_min(neg[:], ret[:], 0.0)
                    en = hpool.tile([128, NS, HID], F32, tag="en")
                    nc.scalar.activation(en[:], neg[:], AF.Exp)
                    nc.vector.tensor_scalar_max(ret[:], ret[:], 0.0)
                    nc.vector.tensor_scalar_add(en[:], en[:], -1.0)
                    nc.vector.tensor_tensor(outs_all[:, mp, :, :], ret[:], en[:], ALU.add)
                    acc_ctx.__exit__(None, None, None)

                # ---------- semantic attention + product MLP ----------
                if c.skip_semantic:
                    zot = keep.tile([128, 2], F32, tag="zot")
                    nc.vector.memset(zot[:], 0.5)
                    for s in range(c.n_slc):
                        nc.sync.dma_start(outd.ap()[s * 128:(s + 1) * 128, :], zot[:])
                sem_mps = [] if c.skip_semantic else list(range(c.n_mp))
                with tc.tile_pool(name="hd_ps", bufs=1, space="PSUM") as hpspool:
                    acc4 = keep.tile([1, c.n_mp], F32, tag="acc4")
                    nc.vector.memset(acc4[:], 0.0)
                    for mp in sem_mps:
                        w1sb = suw1sb if mp < 2 else siw1sb
                        b1rep = SUB1rep if mp < 2 else SIB1rep
                        w2rep = SUW2rep if mp < 2 else SIW2rep
                        for s in range(c.n_slc):
                            o = outs_all[:, mp, s, :]
                            oT_ps = hpspool.tile([HID, 128], F32, space=PSUM, tag="oT_ps", bufs=2)
                            nc.tensor.transpose(oT_ps[:], o, id128[:])
                            oT = hpool.tile([HID, 128], F32, tag="oT")
                            nc.vector.tensor_copy(oT[:], oT_ps[:])
                            tt = hpspool.tile([128, c.AV], F32, space=PSUM, tag="tt", bufs=2)
                            nc.tensor.matmul(out=tt[:], lhsT=oT[:], rhs=w1sb[:],
                                             start=True, stop=True)
                            th = hpool.tile([128, c.AV], F32, tag="th")
                            nc.vector.tensor_tensor(th[:], tt[:], b1rep[:], ALU.add)
                            nc.scalar.activation(th[:], th[:], AF.Tanh)
                            nc.vector.tensor_tensor(th[:], th[:], w2rep[:], ALU.mult)
                            rsum = hpool.tile([128, 1], F32, tag="rsum")
                            nc.vector.tensor_reduce(rsum[:], th[:],
                                                    mybir.AxisListType.X, ALU.add)
                            sp = hpspool.tile([1, 1], F32, space=PSUM, tag="sp")
                            nc.tensor.matmul(out=sp[:], lhsT=rsum[:], rhs=onescol[:],
                                             start=True, stop=True)
                            nc.vector.tensor_tensor(acc4[:, mp:mp + 1],
                                                    acc4[:, mp:mp + 1], sp[:], ALU.add)

                    sem_slcs = [] if c.skip_semantic else list(range(c.n_slc))
                    sin_t = dpool.tile([1, 128], F32, tag="sin")
                    sout_t = dpool.tile([1, 128], F32, tag="sout")
                    zrow = hpool.tile([1, 128], F32, tag="zrow")
                    nc.vector.memset(zrow[:], 0.0)
                    nc.sync.dma_start(sin_t[:], zrow[:])
                    if not c.skip_semantic:
                        nc.sync.dma_start(sin_t[0:1, 0:c.n_mp], acc4[:])
                    nc.gpsimd.collective_compute(
                        "AllReduce", ALU.add,
                        replica_groups=[list(range(c.n_cores))],
                        ins=[sin_t.opt()], outs=[sout_t.opt()],
                    )
                    s4 = hpool.tile([1, c.n_mp], F32, tag="s4")
                    nc.sync.dma_start(s4[:], sout_t[0:1, 0:c.n_mp])
                    e4 = hpool.tile([1, c.n_mp], F32, tag="e4")
                    nc.scalar.activation(e4[:], s4[:], AF.Exp, scale=1.0 / c.B)
                    beta = hpool.tile([1, c.n_mp], F32, tag="beta")
                    for sd in range(2):
                        ssum = hpool.tile([1, 1], F32, tag="ssum")
                        nc.vector.tensor_reduce(ssum[:], e4[:, 2 * sd:2 * sd + 2],
                                                mybir.AxisListType.X, ALU.add)
                        sinv = hpool.tile([1, 1], F32, tag="sinv")
                        nc.vector.reciprocal(sinv[:], ssum[:])
                        nc.vector.tensor_scalar_mul(beta[:, 2 * sd:2 * sd + 2],
                                                    e4[:, 2 * sd:2 * sd + 2], sinv[:])
                    bc_ps = hpspool.tile([128, c.n_mp], F32, space=PSUM, tag="bc_ps")
                    nc.tensor.matmul(out=bc_ps[:], lhsT=ones1[:], rhs=beta[:],
                                     start=True, stop=True)
                    bcol = keep.tile([128, c.n_mp], F32, tag="bcol")
                    nc.vector.tensor_copy(bcol[:], bc_ps[:])

                    for s in sem_slcs:
                        hu = hpool.tile([128, HID], F32, tag="hu")
                        hi_ = hpool.tile([128, HID], F32, tag="hi_")
                        t0 = hpool.tile([128, HID], F32, tag="t0")
                        nc.vector.tensor_scalar_mul(hu[:], outs_all[:, 0, s, :], bcol[:, 0:1])
                        nc.vector.tensor_scalar_mul(t0[:], outs_all[:, 1, s, :], bcol[:, 1:2])
                        nc.vector.tensor_tensor(hu[:], hu[:], t0[:], ALU.add)
                        nc.vector.tensor_scalar_mul(hi_[:], outs_all[:, 2, s, :], bcol[:, 2:3])
                        nc.vector.tensor_scalar_mul(t0[:], outs_all[:, 3, s, :], bcol[:, 3:4])
                        nc.vector.tensor_tensor(hi_[:], hi_[:], t0[:], ALU.add)
                        xx = hpool.tile([128, HID], F32, tag="xx")
                        nc.vector.tensor_tensor(xx[:], hu[:], hi_[:], ALU.mult)
                        xT_ps = hpspool.tile([HID, 128], F32, space=PSUM, tag="xT_ps")
                        nc.tensor.transpose(xT_ps[:], xx[:], id128[:])
                        xT = hpool.tile([HID, 128], F32, tag="xT")
                        nc.vector.tensor_copy(xT[:], xT_ps[:])
                        yy = hpspool.tile([128, c.CH], F32, space=PSUM, tag="yy")
                        nc.tensor.matmul(out=yy[:], lhsT=xT[:], rhs=cw1sb[:],
                                         start=True, stop=True)
                        ya = hpool.tile([128, c.CH], F32, tag="ya")
                        nc.vector.tensor_tensor(ya[:], yy[:], CB1rep[:], ALU.add)
                        nc.vector.tensor_scalar_max(ya[:], ya[:], 0.0)
                        l0t = hpool.tile([128, c.CH], F32, tag="l0t")
                        nc.vector.tensor_tensor(l0t[:], ya[:], CW20rep[:], ALU.mult)
                        l0 = hpool.tile([128, 1], F32, tag="l0")
                        nc.vector.tensor_reduce(l0[:], l0t[:], mybir.AxisListType.X, ALU.add)
                        nc.vector.tensor_tensor(l0t[:], ya[:], CW21rep[:], ALU.mult)
                        l1 = hpool.tile([128, 1], F32, tag="l1")
                        nc.vector.tensor_reduce(l1[:], l0t[:], mybir.AxisListType.X, ALU.add)
                        dl = hpool.tile([128, 1], F32, tag="dl")
                        ot = hpool.tile([128, 2], F32, tag="ot")
                        nc.vector.tensor_tensor(dl[:], l0[:], l1[:], ALU.subtract)
                        nc.scalar.activation(ot[:, 0:1], dl[:], AF.Sigmoid)
                        nc.vector.tensor_tensor(dl[:], l1[:], l0[:], ALU.subtract)
                        nc.scalar.activation(ot[:, 1:2], dl[:], AF.Sigmoid)
                        nc.sync.dma_start(outd.ap()[s * 128:(s + 1) * 128, :], ot[:])

    nc.compile()
    return nc


# ---------------------------------------------------------------------------
# host side: sharding / packing
# ---------------------------------------------------------------------------

def _mp_arrays(inputs, mp):
    if mp < 2:
        return np.asarray(inputs["emi_user"][mp]), np.asarray(inputs["tgt_user"][mp])
    return np.asarray(inputs["emi_item"][mp - 2]), np.asarray(inputs["tgt_item"][mp - 2])


def make_plan(inputs, cfg: Cfg):
    c = cfg
    tcs = np.zeros((c.n_mp, 8, 8), np.int64)
    for mp in range(c.n_mp):
        emi, tgt = _mp_arrays(inputs, mp)
        for k in range(c.n_cores):
            sel = (tgt >= k * c.B_loc) & (tgt < (k + 1) * c.B_loc)
            e, t = emi[sel], tgt[sel] - k * c.B_loc
            cls = ((e[:, 0] >= c.LO).astype(int) + 2 * (e[:, 1] >= c.LO) +
                   4 * (e[:, 2] >= c.LO))
            slc = t // 128
            cnt = np.zeros((8, 8), np.int64)
            np.add.at(cnt, (cls, slc), 1)
            tcs[mp] = np.maximum(tcs[mp], (cnt + 127) // 128)
        for s in range(8):
            if tcs[mp][:, s].sum() == 0:
                tcs[mp][7][s] = 1
    T_raw = int(tcs.sum(axis=(1, 2)).max())
    # choose Tc minimizing padded T, prefer larger chunks
    best = None
    cands = (c.force_Tc,) if c.force_Tc else (20, 22, 24, 26, 28, 30, 32, 36, 40)
    for Tc in cands:
        T_pad = ((T_raw + Tc - 1) // Tc) * Tc
        key = (T_pad, -Tc)
        if best is None or key < best[0]:
            best = (key, Tc, T_pad)
    _, Tc, T = best
    for mp in range(c.n_mp):
        tcs[mp][7][7] += T - tcs[mp].sum()
    return tcs, T, Tc


def _wrap16(vals):
    """[N] values (N % 16 == 0) -> [128, N/16] int16, q7 wrapped layout."""
    v = np.asarray(vals).astype(np.int16).reshape(-1, 16)
    return np.ascontiguousarray(np.tile(v.T, (8, 1)))


def _pack_metapath(emi, tgt, k, c: Cfg, tcs_mp):
    """Pack one (metapath, core) shard grouped by (class, slice).

    Returns (idx16 [3,128,T*8], tlocS [128, T] f32: slice-local target or
    -1e9 for padding)."""
    lo, hi = k * c.B_loc, (k + 1) * c.B_loc
    sel = np.nonzero((tgt >= lo) & (tgt < hi))[0]
    e_all, t_all = emi[sel], tgt[sel] - lo
    cls_all = ((e_all[:, 0] >= c.LO).astype(int) + 2 * (e_all[:, 1] >= c.LO) +
               4 * (e_all[:, 2] >= c.LO))
    slc_all = t_all // 128
    E = c.E_loc
    emi_sh = np.zeros((E, 3), np.int64)
    tlocS = np.full((E,), -1e9, np.float32)
    tpos = 0
    for cl in range(8):
        dummy = np.array([c.LO if (cl >> l) & 1 else 0 for l in range(3)], np.int64)
        for s in range(8):
            ntiles = int(tcs_mp[cl][s])
            if ntiles == 0:
                continue
            seg = np.nonzero((cls_all == cl) & (slc_all == s))[0]
            assert seg.size <= ntiles * 128
            base = tpos * 128
            emi_sh[base:base + seg.size] = e_all[seg]
            emi_sh[base + seg.size:base + ntiles * 128] = dummy
            tlocS[base:base + seg.size] = t_all[seg] - 128 * s
            tpos += ntiles
    assert tpos == c.T
    idx16 = []
    for l in range(3):
        v = emi_sh[:, l].copy()
        v[v >= c.LO] -= c.LO
        idx16.append(_wrap16(v))
    tl = np.ascontiguousarray(tlocS.reshape(c.T, 128).T)
    return np.stack(idx16), tl


def prepare(inputs, cfg: Cfg):
    """Plan and pack all shards. Returns in_maps (one per core)."""
    c = cfg
    tcs, T, Tc = make_plan(inputs, cfg)
    c.tiles_cs = tcs
    c.T = T
    c.Tc = Tc

    f0, f1 = np.asarray(inputs["feats0"]), np.asarray(inputs["feats1"])
    feats_all = np.concatenate([f0, f1], axis=0)
    attn4 = np.stack([np.asarray(inputs["attn_user"][p]).reshape(-1) for p in range(2)] +
                     [np.asarray(inputs["attn_item"][p]).reshape(-1) for p in range(2)])
    rv = np.asarray(inputs["r_vec"])[0].reshape(-1).astype(np.float32)

    in_maps = []
    for k in range(c.n_cores):
        m = {}
        lo_n = k * c.nodes_core
        fs = feats_all[lo_n:lo_n + c.nodes_core]
        pad = c.node_tiles * 128 - c.nodes_core
        if pad:
            fs = np.concatenate([fs, np.zeros((pad, c.F0), np.float32)], axis=0)
        m["featsT"] = np.ascontiguousarray(fs.T, np.float32)
        tw = "0" if lo_n < f0.shape[0] else "1"
        for nm in ("pw", "pb", "w2", "b2", "g", "be"):
            m[nm] = np.asarray(inputs[f"tower{tw}_{nm}"], np.float32)
        m["rvec"] = rv
        m["attn"] = attn4.astype(np.float32)
        emi_l, tl_l = [], []
        for mp in range(c.n_mp):
            emi, tgt = _mp_arrays(inputs, mp)
            et, tl = _pack_metapath(emi, tgt, k, c, tcs[mp])
            emi_l.append(et)
            tl_l.append(tl)
        m["emi16"] = np.concatenate(emi_l).reshape(c.n_mp * 3 * 128, c.T * 8)
        m["tlocs"] = np.concatenate(tl_l).reshape(
            c.n_mp * 128, c.T).astype(ml_dtypes.bfloat16)
        for nm in ("su_w1", "su_b1", "su_w2", "si_w1", "si_b1", "si_w2",
                   "cw1", "cb1", "cw2"):
            m[nm.replace("_", "")] = np.asarray(inputs[nm], np.float32)
        in_maps.append(m)
    return in_maps


# ---------------------------------------------------------------------------
# PJRT SPMD runner (axon path)
# ---------------------------------------------------------------------------


class SpmdRunner:
    def __init__(self, nc, n_cores: int):
        import jax
        from jax.sharding import Mesh, PartitionSpec, NamedSharding
        from jax.experimental.shard_map import shard_map
        from concourse.bass2jax import (
            _bass_exec_p, install_neuronx_cc_hook, partition_id_tensor)

        self.jax = jax
        install_neuronx_cc_hook()
        self.nc = nc
        self.n_cores = n_cores
        partition_name = nc.partition_id_tensor.name if nc.partition_id_tensor else None
        in_names, out_names, out_avals, zero_outs = [], [], [], []
        for alloc in nc.m.functions[0].allocations:
            if not isinstance(alloc, mybir.MemoryLocationSet):
                continue
            name = alloc.memorylocations[0].name
            if alloc.kind == "ExternalInput":
                if name != partition_name:
                    in_names.append(name)
            elif alloc.kind == "ExternalOutput":
                out_names.append(name)
                shape = tuple(alloc.tensor_shape)
                dtype = mybir.dt.np(alloc.dtype)
                out_avals.append(jax.core.ShapedArray(shape, dtype))
                zero_outs.append(np.zeros(shape, dtype))
        self.dbg_name = nc.dbg_addr.name if nc.dbg_addr is not None else None
        n_params = len(in_names)
        in_names = in_names + out_names
        if partition_name is not None:
            in_names.append(partition_name)
        self.in_names, self.out_names = in_names, out_names
        self.n_params, self.out_avals, self.zero_outs = n_params, out_avals, zero_outs

        def _body(*args):
            operands = list(args)
            if partition_name is not None:
                operands.append(partition_id_tensor())
            outs = _bass_exec_p.bind(
                *operands,
                out_avals=tuple(out_avals),
                in_names=tuple(in_names),
                out_names=tuple(out_names),
                lowering_input_output_aliases=(),
                sim_require_finite=True,
                sim_require_nnan=True,
                nc=nc,
            )
            return tuple(outs)

        devices = jax.devices()[:n_cores]
        assert len(devices) == n_cores
        self.mesh = Mesh(np.asarray(devices), ("core",))
        donate = tuple(range(n_params, n_params + len(out_names)))
        in_specs = (PartitionSpec("core"),) * (n_params + len(out_names))
        out_specs = (PartitionSpec("core"),) * len(out_names)
        self.sharded = jax.jit(
            shard_map(_body, mesh=self.mesh, in_specs=in_specs,
                      out_specs=out_specs, check_rep=False),
            donate_argnums=donate, keep_unused=True)
        self.sharding = NamedSharding(self.mesh, PartitionSpec("core"))

    def stage_inputs(self, in_maps):
        jax = self.jax
        if self.dbg_name is not None:
            in_maps = [{**m, self.dbg_name: np.zeros((1, 2), np.uint32)}
                       for m in in_maps]
        staged = []
        for i in range(self.n_params):
            name = self.in_names[i]
            arr = np.concatenate([np.asarray(m[name]) for m in in_maps], axis=0)
            staged.append(jax.device_put(arr, self.sharding))
        jax.block_until_ready(staged)
        self.staged = staged

    def _zeros(self):
        jax = self.jax
        zs = [jax.device_put(
            np.zeros((self.n_cores * z.shape[0], *z.shape[1:]), z.dtype),
            self.sharding) for z in self.zero_outs]
        jax.block_until_ready(zs)
        return zs

    def run(self):
        jax = self.jax
        outs = self.sharded(*self.staged, *self._zeros())
        jax.block_until_ready(outs)
        return [
            {name: np.asarray(outs[i]).reshape(self.n_cores, *self.out_avals[i].shape)[k]
             for i, name in enumerate(self.out_names)}
            for k in range(self.n_cores)
        ]

    def bench(self, iters=20, warmup=3):
        import time
        jax = self.jax
        times = []
        for it in range(warmup + iters):
            zs = self._zeros()
            t0 = time.perf_counter()
            outs = self.sharded(*self.staged, *zs)
            jax.block_until_ready(outs)
            dt = time.perf_counter() - t0
            if it >= warmup:
                times.append(dt)
            del outs
        times = np.array(times)
        return {"min_s": float(times.min()), "med_s": float(np.median(times)),
                "mean_s": float(times.mean()), "n": iters}


_CACHE = {}


def kernel(**inputs) -> np.ndarray:
    cfg = Cfg()
    in_maps = prepare(inputs, cfg)
    key = (cfg.T, cfg.Tc, cfg.tiles_cs.tobytes())
    if key not in _CACHE:
        nc = build_program(cfg)
        _CACHE[key] = (nc, SpmdRunner(nc, cfg.n_cores))
    nc, runner = _CACHE[key]
    runner.stage_inputs(in_maps)
    res = runner.run()
    out = np.empty((cfg.B, 2), np.float32)
    for k in range(cfg.n_cores):
        out[k * cfg.B_loc:(k + 1) * cfg.B_loc] = res[k]["out"]
    return out
